# revision 1
# baseline (speedup 1.0000x reference)
"""Trainium2 Bass kernel for CrossInferBlock (spatial+temporal cross attention
+ out-projection + residual + BatchNorm over (B,T,N)).

Sharding: data-parallel over B across 8 NeuronCores (one batch element per
core). BN batch statistics are all-reduced across cores (8KB collective).

All matmuls run in bf16 (fp32 PSUM accumulate); residual/stats/BN in fp32.
Measured numerics vs the fp32 reference: ~2e-3 relative error.

Device-side token order is ACTOR-MAJOR: tok = j*T + t (j = actor/spatial
index, t = time). Temporal attention groups (8 actors x 16 timesteps) are then
contiguous 128-token slices; spatial attention groups (one timestep, all 128
actors) are single-stride (stride T) slices. Channels/latents live on the
partition axis so every matmul contraction is on partitions and the BN stats
are free-axis reductions.

Schedule: temporal attention is interleaved with the projection token-chunks
(group jg only needs th/ph of chunk jg//4) and initializes stT; spatial
attention is software-pipelined with the g_sp projections after and adds into
stT; the BN stats collective is split in two so the first AllReduce's
cross-core rendezvous overlaps the out-projection tail; BN apply+store is
split across the ACT and DVE engines with bf16 stores on both HWDGE rings
(the host upcasts to fp32).
"""

import sys

if "/opt/trn_rl_repo" not in sys.path:
    sys.path.insert(0, "/opt/trn_rl_repo")

import numpy as np
import ml_dtypes

import concourse.bass as bass
import concourse.bacc as bacc
import concourse.tile as tile
import concourse.mybir as mybir
from concourse.bass_utils import run_bass_kernel_spmd
from contextlib import ExitStack

F32 = mybir.dt.float32
BF16 = mybir.dt.bfloat16
AX = mybir.AxisListType
OP = mybir.AluOpType
ACT_FN = mybir.ActivationFunctionType

N_CORES = 8
B, T, N, C = 8, 16, 128, 1024
L = C // 2            # 512
TOK = T * N           # 2048 tokens per batch element
NTOK_GLOBAL = B * T * N
JG = 8                # actors per temporal group
NGRP = N // JG        # 16 groups
BN_EPS = 1e-5

SP_SCALE = 1.0 / (N * (T + N))   # spatial: /N then /(T+N)
TP_SCALE = 1.0 / (T * (T + N))   # temporal: /T then /(T+N)

NCC = C // 128     # 8 c-chunks
NLC = L // 128     # 4 l-chunks
NTC = TOK // 512   # 4 token chunks

_compiled = None
_last_results = None

USE_COLLECTIVE = True
SINGLE_CC = False


def ts(i, size):
    return bass.ts(i, size)


def _build():
    nc = bacc.Bacc("TRN2", target_bir_lowering=False, debug=False,
                   num_devices=N_CORES)

    # ---- DRAM I/O (token order: actor-major, tok = j*T + t) ----
    xbf_d = nc.dram_tensor("xbf", [C, TOK], BF16, kind="ExternalInput")
    wt_d = nc.dram_tensor("wt", [C, L], BF16, kind="ExternalInput")
    wp_d = nc.dram_tensor("wp", [C, L], BF16, kind="ExternalInput")
    wg_d = nc.dram_tensor("wg", [C, L], BF16, kind="ExternalInput")
    ww_d = nc.dram_tensor("ww", [L, C], BF16, kind="ExternalInput")
    mask_d = nc.dram_tensor("mask", [128, 128], BF16, kind="ExternalInput")
    gb_d = nc.dram_tensor("gb", [128, 16], F32, kind="ExternalInput")
    outy_d = nc.dram_tensor("outy", [C, TOK], BF16, kind="ExternalOutput")

    CT_SPLIT = 2

    with tile.TileContext(nc) as tc:
        with ExitStack() as outer:
            # ---------------- persistent pools ----------------
            cpool = outer.enter_context(tc.tile_pool(name="consts", bufs=1))
            wwpool = outer.enter_context(tc.tile_pool(name="wwp", bufs=1))
            stpool = outer.enter_context(tc.tile_pool(name="stp", bufs=1))
            statpool = outer.enter_context(tc.tile_pool(name="stats", bufs=1))
            pbig = outer.enter_context(
                tc.tile_pool(name="pbig", bufs=1, space="PSUM"))
            psmall = outer.enter_context(
                tc.tile_pool(name="psmall", bufs=1, space="PSUM"))
            drampool = outer.enter_context(
                tc.tile_pool(name="dramp", bufs=1, space="DRAM"))
            xbpool = outer.enter_context(tc.tile_pool(name="xbp", bufs=1))

            mask_sb = cpool.tile([128, 128], BF16, name="mask_sb",
                                 tag="mask_sb")
            gb_sb = cpool.tile([128, 16], F32, name="gb_sb", tag="gb_sb")
            ww_all = wwpool.tile([128, NLC * C], BF16, name="ww_all", tag="ww")
            stT = stpool.tile([128, NLC * TOK], BF16, name="stT", tag="stT")

            stat_sum = statpool.tile([128, 32], F32, name="stat_sum",
                                     tag="stat_sum")
            stat_sq = statpool.tile([128, 32], F32, name="stat_sq",
                                    tag="stat_sq")
            red_in = statpool.tile([128, 16], F32, name="red_in", tag="red_in")
            red_out = statpool.tile([128, 16], F32, name="red_out",
                                    tag="red_out")
            scalev = statpool.tile([128, 8], F32, name="scalev", tag="scalev")
            biasv = statpool.tile([128, 8], F32, name="biasv", tag="biasv")

            cc_in = drampool.tile([128, 4], F32, name="cc_in", tag="cc_in")
            cc_out = drampool.tile([128, 4], F32, name="cc_out", tag="cc_out")
            cc_in2 = drampool.tile([128, 12], F32, name="cc_in2", tag="cc_in2")
            cc_big_in = drampool.tile([128, 16], F32, name="cc_big_in",
                                      tag="cc_big_in")
            cc_big_out = drampool.tile([128, 16], F32, name="cc_big_out",
                                       tag="cc_big_out")
            cc_warm_in = drampool.tile([128, 1], F32, name="cc_warm_in",
                                       tag="cc_warm_in")
            cc_warm_out = drampool.tile([128, 1], F32, name="cc_warm_out",
                                        tag="cc_warm_out")
            cc_out2 = drampool.tile([128, 12], F32, name="cc_out2",
                                    tag="cc_out2")

            def sp_view(tile_ap, i):
                """[128, TOK] tile -> all 128 actors at time i (stride T)."""
                return tile_ap.rearrange("p (j t) -> p t j", t=T)[:, i:i + 1, :]

            with ExitStack() as mid:
                thpool = mid.enter_context(tc.tile_pool(name="thp", bufs=1))
                gpool = mid.enter_context(tc.tile_pool(name="gp", bufs=1))
                attnpool = mid.enter_context(tc.tile_pool(name="attn", bufs=1))

                thT = [thpool.tile([128, TOK], BF16, name=f"thT{lc}",
                                   tag=f"thT{lc}") for lc in range(NLC)]
                phT = [thpool.tile([128, TOK], BF16, name=f"phT{lc}",
                                   tag=f"phT{lc}") for lc in range(NLC)]
                g_sp = [gpool.tile([128, L], BF16, name=f"gsp{i}",
                                   tag=f"gsp{i}") for i in range(T)]
                g_act = [gpool.tile([128, L], BF16, name=f"gact{j}",
                                    tag=f"gact{j}") for j in range(NGRP)]

                with ExitStack() as phase_a:
                    wpool = phase_a.enter_context(
                        tc.tile_pool(name="wp", bufs=1))

                    # x as one tile, free = (c, tok); chunked 1MiB DMAs on the
                    # sync HWDGE ring so the first matmuls start early.
                    # Outer-level pool: the bf16 x also serves as the residual
                    # input during the out-projection.
                    xbf = xbpool.tile([128, NCC * TOK], BF16, name="xbf",
                                      tag="xbf")
                    xbf_src = xbf_d.rearrange("(a p) k -> p a k", a=NCC)
                    xbf_dst = xbf.rearrange("p (a k) -> p a k", a=NCC)
                    wt_all = wpool.tile([128, NCC * L], BF16, name="wt_all",
                                        tag="wt")
                    wp_all = wpool.tile([128, NCC * L], BF16, name="wp_all",
                                        tag="wp")
                    wg_all = wpool.tile([128, NCC * L], BF16, name="wg_all",
                                        tag="wg")

                    nc.sync.dma_start(xbf_dst[:, :, ts(0, 512)],
                                      xbf_src[:, :, ts(0, 512)])
                    nc.gpsimd.dma_start(
                        wt_all.rearrange("p (a l) -> p a l", a=NCC),
                        wt_d.rearrange("(a p) l -> p a l", a=NCC))
                    nc.sync.dma_start(xbf_dst[:, :, ts(1, 512)],
                                      xbf_src[:, :, ts(1, 512)])
                    nc.gpsimd.dma_start(
                        wp_all.rearrange("p (a l) -> p a l", a=NCC),
                        wp_d.rearrange("(a p) l -> p a l", a=NCC))
                    nc.sync.dma_start(xbf_dst[:, :, ts(2, 512)],
                                      xbf_src[:, :, ts(2, 512)])
                    nc.sync.dma_start(xbf_dst[:, :, ts(3, 512)],
                                      xbf_src[:, :, ts(3, 512)])
                    nc.gpsimd.dma_start(
                        wg_all.rearrange("p (a l) -> p a l", a=NCC),
                        wg_d.rearrange("(a p) l -> p a l", a=NCC))
                    nc.gpsimd.dma_start(mask_sb[:], mask_d[:])
                    nc.gpsimd.dma_start(gb_sb[:], gb_d[:])
                    nc.gpsimd.dma_start(
                        ww_all.rearrange("p (a c1) -> p a c1", a=NLC),
                        ww_d.rearrange("(a p) c1 -> p a c1", a=NLC))
                    if USE_COLLECTIVE:
                        # warm-up collective: pays the CC ring's one-time
                        # setup + initial core rendezvous during the DMA load
                        # phase so the BN collectives at the tail are cheap
                        nc.gpsimd.dma_start(cc_warm_in[:], gb_d[:, 0:1])
                        nc.gpsimd.collective_compute(
                            "AllReduce", OP.add,
                            replica_groups=[list(range(N_CORES))],
                            ins=[cc_warm_in.opt()], outs=[cc_warm_out.opt()])

                    def xsl(c, lo, n):
                        return xbf[:, c * TOK + lo:c * TOK + lo + n]

                    def wsl(w, c, lc):
                        return w[:, c * L + lc * 128:c * L + (lc + 1) * 128]

                    # temporal attention; INITIALIZES stT (scaled copy, ACT)
                    pend_tp = []   # (jg, twp)

                    def emit_tw(jg):
                        twp = psmall.tile([128, 128], F32, name="ps_tw",
                                          tag="ps_small", bufs=4)
                        for lc in range(NLC):
                            nc.tensor.matmul(twp[:], phT[lc][:, ts(jg, 128)],
                                             thT[lc][:, ts(jg, 128)],
                                             start=(lc == 0),
                                             stop=(lc == NLC - 1))
                        pend_tp.append((jg, twp))

                    def emit_tp():
                        jg, twp = pend_tp.pop(0)
                        sb = attnpool.tile([128, 128], BF16, name="sb",
                                           tag="sb", bufs=3)
                        nc.vector.tensor_mul(sb[:], twp[:], mask_sb[:])
                        pp = psmall.tile([128, 512], F32, name="ps_tp",
                                         tag="ps_small", bufs=4)
                        for lc in range(NLC):
                            nc.tensor.matmul(pp[:, ts(lc, 128)],
                                             g_act[jg][:, ts(lc, 128)], sb[:])
                        dst = stT.rearrange("p (a k) -> p a k", a=NLC)[
                            :, :, ts(jg, 128)]
                        src = pp.rearrange("p (a k) -> p a k", a=NLC)
                        nc.scalar.mul(dst, src, TP_SCALE)

                    # ------- phase 1: projections + g_act + temporal -------
                    for tck in range(NTC):
                        for (w_all, dst) in ((wt_all, thT), (wp_all, phT)):
                            for lc in range(NLC):
                                ps = pbig.tile([128, 512], F32, name="ps_proj",
                                               tag="ps_big", bufs=4)
                                for c in range(NCC):
                                    nc.tensor.matmul(
                                        ps[:], wsl(w_all, c, lc),
                                        xsl(c, tck * 512, 512),
                                        start=(c == 0), stop=(c == NCC - 1))
                                nc.vector.tensor_copy(
                                    dst[lc][:, ts(tck, 512)], ps[:])
                        for jg in range(4 * tck, 4 * tck + 4):
                            ps = pbig.tile([128, 512], F32, name="ps_ga",
                                           tag="ps_big", bufs=4)
                            for c in range(NCC):
                                nc.tensor.matmul(
                                    ps[:], xsl(c, jg * 128, 128),
                                    wg_all[:, ts(c, 512)],
                                    start=(c == 0), stop=(c == NCC - 1))
                            nc.vector.tensor_copy(g_act[jg][:], ps[:])
                            emit_tw(jg)
                            if len(pend_tp) >= 2:
                                emit_tp()
                    while pend_tp:
                        emit_tp()

                    # ------- phase 2: g_sp + spatial attention (ADD) -------
                    pend_sp = []   # (i, swp)

                    def emit_gsp(i):
                        ps = pbig.tile([128, 512], F32, name="ps_g",
                                       tag="ps_big", bufs=4)
                        for c in range(NCC):
                            nc.tensor.matmul(
                                ps[:],
                                sp_view(xbf[:, c * TOK:(c + 1) * TOK], i),
                                wg_all[:, ts(c, 512)],
                                start=(c == 0), stop=(c == NCC - 1))
                        nc.scalar.copy(g_sp[i][:], ps[:])

                    def emit_sw(i):
                        swp = psmall.tile([128, 128], F32, name="ps_sw",
                                          tag="ps_small", bufs=4)
                        for lc in range(NLC):
                            nc.tensor.matmul(swp[:], sp_view(phT[lc], i),
                                             sp_view(thT[lc], i),
                                             start=(lc == 0),
                                             stop=(lc == NLC - 1))
                        pend_sp.append((i, swp))

                    def emit_sp_from(i, swb_ap):
                        pp = psmall.tile([128, 512], F32, name="ps_sp",
                                         tag="ps_small", bufs=4)
                        for lc in range(NLC):
                            nc.tensor.matmul(pp[:, ts(lc, 128)],
                                             g_sp[i][:, ts(lc, 128)], swb_ap)
                        # read-modify-write add into stT, strided dest
                        dst = stT.rearrange("p (a j t) -> p t a j",
                                            t=T, a=NLC)[:, i, :, :]
                        src = pp.rearrange("p (a j) -> p a j", a=NLC)
                        nc.vector.scalar_tensor_tensor(
                            out=dst, in0=src, scalar=SP_SCALE, in1=dst,
                            op0=OP.mult, op1=OP.add)

                    def emit_sp():
                        i, swp = pend_sp.pop(0)
                        swb = attnpool.tile([128, 128], BF16, name="swb",
                                            tag="swb", bufs=3)
                        nc.vector.tensor_copy(swb[:], swp[:])
                        emit_sp_from(i, swb[:])

                    # pre-compute the last two sw groups (they only need
                    # thT/phT) so the pipeline tail has no PE->DVE handoffs
                    swb_pre = {}
                    for i in (T - 2, T - 1):
                        swp = psmall.tile([128, 128], F32, name="ps_swp",
                                          tag="ps_small", bufs=4)
                        for lc in range(NLC):
                            nc.tensor.matmul(swp[:], sp_view(phT[lc], i),
                                             sp_view(thT[lc], i),
                                             start=(lc == 0),
                                             stop=(lc == NLC - 1))
                        pre = attnpool.tile([128, 128], BF16,
                                            name=f"swbpre{i}",
                                            tag=f"swbpre{i}", bufs=1)
                        nc.vector.tensor_copy(pre[:], swp[:])
                        swb_pre[i] = pre

                    for i in range(T):
                        emit_gsp(i)
                        if 1 <= i < T - 1:
                            emit_sw(i - 1)
                        if i >= 2:
                            emit_sp()
                    while pend_sp:
                        emit_sp()
                    emit_sp_from(T - 2, swb_pre[T - 2][:])
                    emit_sp_from(T - 1, swb_pre[T - 1][:])

            # ------- phase 3: out-projection + residual + stats -------
            with tc.tile_pool(name="outp", bufs=1) as outpool, \
                 tc.tile_pool(name="yp", bufs=1) as ypool, \
                 tc.tile_pool(name="sqp", bufs=1) as sqpool:
                out_sb = []
                inv_n = 1.0 / float(NTOK_GLOBAL)

                def emit_outproj(ct):
                    o = outpool.tile([128, TOK], BF16, name=f"out{ct}",
                                     tag=f"out{ct}")
                    out_sb.append(o)
                    for tck in range(NTC):
                        ps = pbig.tile([128, 512], F32, name="ps_out",
                                       tag="ps_big", bufs=4)
                        for lc in range(NLC):
                            nc.tensor.matmul(
                                ps[:],
                                ww_all[:, lc * C + ct * 128:
                                       lc * C + (ct + 1) * 128],
                                stT[:, lc * TOK + tck * 512:
                                    lc * TOK + tck * 512 + 512],
                                start=(lc == 0), stop=(lc == NLC - 1))
                        col = ct * NTC + tck
                        nc.vector.scalar_tensor_tensor(
                            out=o[:, ts(tck, 512)], in0=ps[:], scalar=1.0,
                            in1=xbf[:, ct * TOK + tck * 512:
                                    ct * TOK + tck * 512 + 512],
                            op0=OP.mult, op1=OP.add,
                            accum_out=stat_sum[:, col:col + 1])
                        sq = sqpool.tile([128, 512], F32, name="sqscr",
                                         tag="sq", bufs=3)
                        nc.scalar.activation(
                            sq[:], o[:, ts(tck, 512)], ACT_FN.Square,
                            accum_out=stat_sq[:, col:col + 1])

                def emit_stats_cc(ct_lo, ct_hi, cci, cco):
                    """AllReduce sum/sumsq for channel tiles [ct_lo, ct_hi)."""
                    n = ct_hi - ct_lo
                    nc.vector.tensor_reduce(
                        red_in[:, ct_lo:ct_hi],
                        stat_sum.rearrange("p (a b) -> p a b", a=8)[
                            :, ct_lo:ct_hi, :],
                        axis=AX.X, op=OP.add)
                    nc.vector.tensor_reduce(
                        red_in[:, 8 + ct_lo:8 + ct_hi],
                        stat_sq.rearrange("p (a b) -> p a b", a=8)[
                            :, ct_lo:ct_hi, :],
                        axis=AX.X, op=OP.add)
                    if USE_COLLECTIVE:
                        nc.gpsimd.dma_start(cci[:, 0:n],
                                            red_in[:, ct_lo:ct_hi])
                        nc.gpsimd.dma_start(cci[:, n:2 * n],
                                            red_in[:, 8 + ct_lo:8 + ct_hi])
                        nc.gpsimd.collective_compute(
                            "AllReduce", OP.add,
                            replica_groups=[list(range(N_CORES))],
                            ins=[cci[:, 0:2 * n].opt()],
                            outs=[cco[:, 0:2 * n].opt()])
                        nc.gpsimd.dma_start(red_out[:, ct_lo:ct_hi],
                                            cco[:, 0:n])
                        nc.gpsimd.dma_start(red_out[:, 8 + ct_lo:8 + ct_hi],
                                            cco[:, n:2 * n])
                    else:
                        nc.vector.tensor_scalar_mul(
                            red_out[:, ct_lo:ct_hi], red_in[:, ct_lo:ct_hi],
                            float(N_CORES))
                        nc.vector.tensor_scalar_mul(
                            red_out[:, 8 + ct_lo:8 + ct_hi],
                            red_in[:, 8 + ct_lo:8 + ct_hi], float(N_CORES))

                def emit_bn_params(part, lo, hi):
                    n = hi - lo
                    mean = statpool.tile([128, n], F32, name=f"mean{part}",
                                         tag=f"mean{part}")
                    var = statpool.tile([128, n], F32, name=f"var{part}",
                                        tag=f"var{part}")
                    std = statpool.tile([128, n], F32, name=f"std{part}",
                                        tag=f"std{part}")
                    rstd = statpool.tile([128, n], F32, name=f"rstd{part}",
                                         tag=f"rstd{part}")
                    nc.vector.tensor_scalar_mul(mean[:], red_out[:, lo:hi],
                                                inv_n)
                    nc.vector.tensor_scalar_mul(var[:],
                                                red_out[:, 8 + lo:8 + hi],
                                                inv_n)
                    nc.vector.tensor_mul(std[:], mean[:], mean[:])
                    nc.vector.tensor_tensor(var[:], var[:], std[:],
                                            op=OP.subtract)
                    nc.vector.tensor_scalar_add(var[:], var[:], BN_EPS)
                    nc.scalar.activation(std[:], var[:], ACT_FN.Sqrt, bias=0.0)
                    nc.vector.reciprocal(rstd[:], std[:])
                    nc.vector.tensor_mul(scalev[:, lo:hi], rstd[:],
                                         gb_sb[:, lo:hi])
                    nc.vector.tensor_mul(rstd[:], mean[:], scalev[:, lo:hi])
                    nc.vector.tensor_tensor(biasv[:, lo:hi],
                                            gb_sb[:, 8 + lo:8 + hi], rstd[:],
                                            op=OP.subtract)

                def emit_apply(ct):
                    # DVE gets 5 channel-tiles, ACT 3; stores on the sync and
                    # scalar HWDGE rings; bf16 stores (host upcasts)
                    on_act = ct in (1, 4, 7)
                    for h in range(2):
                        src = out_sb[ct][:, ts(h, 1024)]
                        if on_act:
                            y = ypool.tile([128, 1024], BF16, name="ya",
                                           tag="ya", bufs=6)
                            nc.scalar.activation(
                                y[:], src, ACT_FN.Identity,
                                scale=scalev[:, ct:ct + 1],
                                bias=biasv[:, ct:ct + 1])
                            nc.scalar.dma_start(
                                outy_d[ts(ct, 128), ts(h, 1024)], y[:])
                        else:
                            y = ypool.tile([128, 1024], BF16, name="yb",
                                           tag="yb", bufs=6)
                            nc.vector.tensor_scalar(
                                out=y[:], in0=src,
                                scalar1=scalev[:, ct:ct + 1],
                                scalar2=biasv[:, ct:ct + 1],
                                op0=OP.mult, op1=OP.add)
                            nc.sync.dma_start(
                                outy_d[ts(ct, 128), ts(h, 1024)], y[:])

                if SINGLE_CC:
                    for ct in range(NCC):
                        emit_outproj(ct)
                    emit_stats_cc(0, NCC, cc_big_in, cc_big_out)
                    emit_bn_params(0, 0, NCC)
                    for ct in range(NCC):
                        emit_apply(ct)
                else:
                    for ct in range(CT_SPLIT):
                        emit_outproj(ct)
                    # early collective: covers channel tiles [0, CT_SPLIT);
                    # its rendezvous overlaps the remaining out-projection and
                    # aligns the cores so the second collective is cheap
                    emit_stats_cc(0, CT_SPLIT, cc_in, cc_out)
                    for ct in range(CT_SPLIT, NCC):
                        emit_outproj(ct)
                    emit_stats_cc(CT_SPLIT, NCC, cc_in2, cc_out2)
                    emit_bn_params(0, 0, CT_SPLIT)
                    for ct in range(CT_SPLIT):
                        emit_apply(ct)
                    emit_bn_params(1, CT_SPLIT, NCC)
                    for ct in range(CT_SPLIT, NCC):
                        emit_apply(ct)

    nc.compile()
    return nc




def _get_compiled():
    global _compiled
    if _compiled is None:
        _compiled = _build()
    return _compiled


def kernel(x, Wt, Wp, Wg, Ww, gamma, beta, _trace=False, _trace_kwargs=None):
    global _last_results
    nc = _get_compiled()

    x = np.asarray(x, dtype=np.float32)
    Wt = np.asarray(Wt, dtype=np.float32)
    Wp = np.asarray(Wp, dtype=np.float32)
    Wg = np.asarray(Wg, dtype=np.float32)
    Ww = np.asarray(Ww, dtype=np.float32)
    gamma = np.asarray(gamma, dtype=np.float32)
    beta = np.asarray(beta, dtype=np.float32)

    bf = ml_dtypes.bfloat16
    wt_t = np.ascontiguousarray(Wt.T).astype(bf)      # [C, L]
    wp_t = np.ascontiguousarray(Wp.T).astype(bf)
    wg_t = np.ascontiguousarray(Wg.T).astype(bf)
    ww_t = np.ascontiguousarray(Ww.T).astype(bf)      # [L, C]
    r = np.arange(128)
    mask = (r[:, None] // T == r[None, :] // T).astype(bf)
    gb = np.concatenate(
        [gamma.reshape(NCC, 128).T,
         beta.reshape(NCC, 128).T], axis=1).astype(np.float32)  # [128, 16]

    # actor-major token order: tok = j*T + t
    xa = x.transpose(0, 2, 1, 3).reshape(B, TOK, C)
    in_maps = []
    for b in range(B):
        xT = np.ascontiguousarray(xa[b].T)            # [C, TOK] f32
        in_maps.append(dict(
            xbf=xT.astype(bf), wt=wt_t, wp=wp_t, wg=wg_t, ww=ww_t,
            mask=mask, gb=gb))

    res = run_bass_kernel_spmd(nc, in_maps, list(range(N_CORES)),
                               trace=_trace, **(_trace_kwargs or {}))
    _last_results = res

    ys = []
    for b in range(B):
        o = np.asarray(res.results[b]["outy"], dtype=np.float32)   # [C, TOK]
        ys.append(o.T.reshape(N, T, C).transpose(1, 0, 2))          # [T, N, C]
    return np.stack(ys)

